# revision 1
# baseline (speedup 1.0000x reference)
"""Trainium2 Bass kernel v2 for nn_Decoder — col-tiled recurrence.

Key differences vs baseline:
- Gate matmuls col-tiled across PSUM strips (r@0,z@32,hn@64,in@96) for PE
  concurrency; biases/x-parts injected into PSUM via tiny I4/ones matmuls.
- No DRAM round-trip transposes: PE is_transpose for c/h0/h1 (on-chip).
- Single activation table (sigmoid set) for the whole recurrence: exp(x)
  computed as sigmoid(x)/(1-sigmoid(x)); phase B uses the exp/ln set once.
- Phase B: weight tile streamed once (v-outer), exp-accum online, per-row
  log-sum-exp without max subtraction, multi-engine normalize tail.
"""

import sys

sys.path.insert(0, "/opt/trn_rl_repo")

import numpy as np
import ml_dtypes

import concourse.bass as bass
import concourse.bacc as bacc
import concourse.tile as tile
from concourse import mybir
from contextlib import ExitStack

F32 = mybir.dt.float32
BF16 = mybir.dt.bfloat16
FP8 = mybir.dt.float8e4
SW, SF = 256.0, 16.0  # fp8 scales for out_W / feats
AF = mybir.ActivationFunctionType
ALU = mybir.AluOpType

BL = 4
NCORES = 8


class Cfg:
    def __init__(self, T=64, S=512, H=512, IN=256, V=32000, VT=500):
        self.T, self.S, self.H, self.IN, self.V, self.VT = T, S, H, IN, V, VT
        self.st = S // 128
        self.hkt = H // 128
        self.xkt = IN // 128
        self.nvt = V // VT
        self.rows = T * BL
        self.mch = self.rows // 128
        assert S % 128 == 0 and H % 128 == 0 and IN % 128 == 0
        assert V % VT == 0 and VT <= 512 and self.rows % 128 == 0


def build(cfg: Cfg, reps: int = 1):
    T, S, H, IN, V, VT = cfg.T, cfg.S, cfg.H, cfg.IN, cfg.V, cfg.VT
    st, hkt, xkt, nvt, mch = cfg.st, cfg.hkt, cfg.xkt, cfg.nvt, cfg.mch
    G2 = 2 * H

    nc = bacc.Bacc()

    # ---- DRAM I/O ----
    d_hid = nc.dram_tensor("hid", [128, BL, st, H], BF16, kind="ExternalInput")
    d_xT = nc.dram_tensor("xT", [128, xkt, cfg.rows], BF16, kind="ExternalInput")
    d_maskT = nc.dram_tensor("maskT", [128, st, BL], F32, kind="ExternalInput")
    d_enclT = nc.dram_tensor("enclT", [128, st, BL], F32, kind="ExternalInput")
    d_aWhT = nc.dram_tensor("aWhT", [128, hkt, 1], BF16, kind="ExternalInput")
    d_hT0 = nc.dram_tensor("hT0", [128, 2 * hkt, BL], BF16, kind="ExternalInput")
    d_hnat0 = nc.dram_tensor("hnat0", [2, BL, H], F32, kind="ExternalInput")
    d_w0i_rz = nc.dram_tensor("w0i_rz", [128, xkt + hkt, G2], BF16, kind="ExternalInput")
    d_w0i_n = nc.dram_tensor("w0i_n", [128, xkt + hkt, H], BF16, kind="ExternalInput")
    d_w0h_rz = nc.dram_tensor("w0h_rz", [128, hkt, G2], BF16, kind="ExternalInput")
    d_w0h_n = nc.dram_tensor("w0h_n", [128, hkt, H], BF16, kind="ExternalInput")
    d_w1i_rz = nc.dram_tensor("w1i_rz", [128, hkt, G2], BF16, kind="ExternalInput")
    d_w1i_n = nc.dram_tensor("w1i_n", [128, hkt, H], BF16, kind="ExternalInput")
    d_w1h_rz = nc.dram_tensor("w1h_rz", [128, hkt, G2], BF16, kind="ExternalInput")
    d_w1h_n = nc.dram_tensor("w1h_n", [128, hkt, H], BF16, kind="ExternalInput")
    d_brz0 = nc.dram_tensor("brz0", [1, G2], BF16, kind="ExternalInput")
    d_bin0 = nc.dram_tensor("bin0", [1, H], BF16, kind="ExternalInput")
    d_bhn0 = nc.dram_tensor("bhn0", [1, H], BF16, kind="ExternalInput")
    d_brz1 = nc.dram_tensor("brz1", [1, G2], BF16, kind="ExternalInput")
    d_bin1 = nc.dram_tensor("bin1", [1, H], BF16, kind="ExternalInput")
    d_bhn1 = nc.dram_tensor("bhn1", [1, H], BF16, kind="ExternalInput")
    d_sel = nc.dram_tensor("sel", [st * BL, BL], BF16, kind="ExternalInput")
    d_eyeb = nc.dram_tensor("eyeb", [128, BL], BF16, kind="ExternalInput")
    d_eyef = nc.dram_tensor("eyef", [128, BL], F32, kind="ExternalInput")
    d_outWT = nc.dram_tensor("outWT", [128, 2 * hkt, V], FP8, kind="ExternalInput")
    d_outb = nc.dram_tensor("outb", [1, V], BF16, kind="ExternalInput")
    d_out = nc.dram_tensor("out", [cfg.rows, V], F32, kind="ExternalOutput")

    with tile.TileContext(nc) as tc, ExitStack() as octx:
        keep = octx.enter_context(tc.tile_pool(name="keep", bufs=1))
        featsT = keep.tile([128, 2 * hkt, cfg.rows], BF16)
        ones_1_128 = keep.tile([1, 128], BF16)
        nc.vector.memset(ones_1_128[:], 1.0)

        def one_rep():
            with ExitStack() as actx:
                sing = actx.enter_context(tc.tile_pool(name="sing", bufs=1))
                work = actx.enter_context(tc.tile_pool(name="work", bufs=2))
                hpool = actx.enter_context(tc.tile_pool(name="hpool", bufs=2))
                gxp = actx.enter_context(tc.tile_pool(name="gxp", bufs=2))
                dpool = actx.enter_context(tc.tile_pool(name="dram", bufs=1, space="DRAM"))
                ps_g0p = actx.enter_context(tc.tile_pool(name="ps_g0", bufs=1, space="PSUM"))
                ps_g1p = actx.enter_context(tc.tile_pool(name="ps_g1", bufs=1, space="PSUM"))
                ps_cp = actx.enter_context(tc.tile_pool(name="ps_c", bufs=1, space="PSUM"))
                ps_smf = actx.enter_context(tc.tile_pool(name="ps_smf", bufs=2, space="PSUM"))
                ps_smb = actx.enter_context(tc.tile_pool(name="ps_smb", bufs=1, space="PSUM"))

                # ---- resident loads ----
                hid = sing.tile([128, BL, st, H], BF16)
                nc.sync.dma_start(hid[:], d_hid[:])
                xT = sing.tile([128, xkt, cfg.rows], BF16)
                nc.sync.dma_start(xT[:], d_xT[:])
                maskT = sing.tile([128, st, BL], F32)
                nc.sync.dma_start(maskT[:], d_maskT[:])
                enclT = sing.tile([128, st, BL], F32)
                nc.sync.dma_start(enclT[:], d_enclT[:])
                aWhT = sing.tile([128, hkt, 1], BF16)
                nc.sync.dma_start(aWhT[:], d_aWhT[:])
                hT_init = sing.tile([128, 2 * hkt, BL], BF16)
                nc.sync.dma_start(hT_init[:], d_hT0[:])
                w0i_rz = sing.tile([128, xkt + hkt, G2], BF16)
                nc.sync.dma_start(w0i_rz[:], d_w0i_rz[:])
                w0i_n = sing.tile([128, xkt + hkt, H], BF16)
                nc.sync.dma_start(w0i_n[:], d_w0i_n[:])
                w0h_rz = sing.tile([128, hkt, G2], BF16)
                nc.sync.dma_start(w0h_rz[:], d_w0h_rz[:])
                w0h_n = sing.tile([128, hkt, H], BF16)
                nc.sync.dma_start(w0h_n[:], d_w0h_n[:])
                w1i_rz = sing.tile([128, hkt, G2], BF16)
                nc.sync.dma_start(w1i_rz[:], d_w1i_rz[:])
                w1i_n = sing.tile([128, hkt, H], BF16)
                nc.sync.dma_start(w1i_n[:], d_w1i_n[:])
                w1h_rz = sing.tile([128, hkt, G2], BF16)
                nc.sync.dma_start(w1h_rz[:], d_w1h_rz[:])
                w1h_n = sing.tile([128, hkt, H], BF16)
                nc.sync.dma_start(w1h_n[:], d_w1h_n[:])
                brz0 = sing.tile([1, G2], BF16)
                nc.sync.dma_start(brz0[:], d_brz0[:])
                bin0 = sing.tile([1, H], BF16)
                nc.sync.dma_start(bin0[:], d_bin0[:])
                bhn0 = sing.tile([1, H], BF16)
                nc.sync.dma_start(bhn0[:], d_bhn0[:])
                brz1 = sing.tile([1, G2], BF16)
                nc.sync.dma_start(brz1[:], d_brz1[:])
                bin1 = sing.tile([1, H], BF16)
                nc.sync.dma_start(bin1[:], d_bin1[:])
                bhn1 = sing.tile([1, H], BF16)
                nc.sync.dma_start(bhn1[:], d_bhn1[:])
                sel = sing.tile([st * BL, BL], BF16)
                nc.sync.dma_start(sel[:], d_sel[:])
                eyeb = sing.tile([128, BL], BF16)
                nc.sync.dma_start(eyeb[:], d_eyeb[:])
                eyef = sing.tile([128, BL], F32)
                nc.sync.dma_start(eyef[:], d_eyef[:])

                ones_1_4 = sing.tile([1, BL], BF16)
                nc.vector.memset(ones_1_4[:], 1.0)
                ones_128_1 = sing.tile([128, 1], BF16)
                nc.vector.memset(ones_128_1[:], 1.0)
                ones_f16 = sing.tile([128, st * BL], F32)
                nc.vector.memset(ones_f16[:], 1.0)
                ones_h = sing.tile([36, 512], F32)
                nc.vector.memset(ones_h[:], 1.0)

                h0nat_init = sing.tile([36, H], F32)
                nc.sync.dma_start(h0nat_init[32:36, :], d_hnat0[0, :, :])
                h1nat_init = sing.tile([36, H], F32)
                nc.sync.dma_start(h1nat_init[32:36, :], d_hnat0[1, :, :])

                # ---- hoisted psum tiles (ping-pong) ----
                g0a = ps_g0p.tile([128, 512], F32, tag="g0a", name="g0a")
                g0b = ps_g0p.tile([128, 512], F32, tag="g0b", name="g0b")
                g1a = ps_g1p.tile([128, 512], F32, tag="g1a", name="g1a")
                g1b = ps_g1p.tile([128, 512], F32, tag="g1b", name="g1b")
                g0bufs, g1bufs = [g0a, g0b], [g1a, g1b]
                ps_c = ps_cp.tile([128, 512], F32, tag="c")
                for tl in (g0a, g0b, g1a, g1b, ps_c):
                    nc.vector.memset(tl[:], 0.0)

                # ---- gi0x precompute: x-part of L0 gates (+brz0/bin0 biases) ----
                # DRAM layout [rows, 3, 512]; chunk n: 0=r, 1=z, 2=n-gate
                gi0x_dram = dpool.tile([cfg.rows, 3, 512], BF16, tag="gi0x")
                for m in range(mch):
                    for n in range(3):
                        acc = g0bufs[(m * 3 + n) % 2]
                        for kt in range(xkt):
                            rhs = (w0i_rz[:, kt, n * 512:(n + 1) * 512] if n < 2
                                   else w0i_n[:, kt, :])
                            nc.tensor.matmul(
                                acc[:], xT[:, kt, m * 128:(m + 1) * 128], rhs,
                                start=(kt == 0), stop=False)
                        brow = brz0[:, n * 512:(n + 1) * 512] if n < 2 else bin0[:]
                        nc.tensor.matmul(acc[:], ones_1_128[:], brow,
                                         start=False, stop=True)
                        gxs = work.tile([128, 512], BF16, tag="gxs", bufs=2)
                        nc.vector.tensor_copy(gxs[:], acc[:])
                        nc.sync.dma_start(
                            gi0x_dram[m * 128:(m + 1) * 128, n, :], gxs[:])

                # ---- recurrence ----
                h0T_prev = hT_init[:, 0:hkt, :]
                h0n_prev = h0nat_init
                h1n_prev = h1nat_init

                gx_t = gxp.tile([BL, 3, 512], BF16, tag="gx")
                nc.sync.dma_start(gx_t[:], gi0x_dram[0:BL, :, :])

                for t in range(T):
                    tc0, tc1 = t * BL, (t + 1) * BL
                    h1T_prev = (hT_init[:, hkt:2 * hkt, :] if t == 0
                                else featsT[:, 0:hkt, (t - 1) * BL:t * BL])

                    # prefetch next step's gx
                    gx_cur = gx_t
                    if t + 1 < T:
                        gx_t = gxp.tile([BL, 3, 512], BF16, tag="gx")
                        nc.sync.dma_start(gx_t[:],
                                          gi0x_dram[(t + 1) * BL:(t + 2) * BL, :, :])

                    # === PE: s = aWh . h1_prev -> [1, BL] ===
                    ps_small = ps_smf.tile([128, 64], F32, tag="sm")
                    ps_s = ps_small[0:1, 0:BL]
                    for kt in range(hkt):
                        nc.tensor.matmul(ps_s[:], aWhT[:, kt, :], h1T_prev[:, kt, :],
                                         start=(kt == 0), stop=(kt == hkt - 1))
                    s_sb = work.tile([1, BL], BF16, tag="s_sb")
                    nc.vector.tensor_copy(s_sb[:], ps_s[:])
                    ps_sbc = ps_small[:, 4:4 + BL]
                    nc.tensor.matmul(ps_sbc, ones_1_128[:], s_sb[:],
                                     start=True, stop=True)

                    # === PE: inject gx (x-part+biases) and biases into gate psums
                    ps_g0 = g0bufs[t % 2]
                    nc.tensor.matmul(ps_g0[0:4, :], eyeb[0:4, :], gx_cur[0:4, 0, :],
                                     start=True, stop=False, tile_position=(0, 0))
                    nc.tensor.matmul(ps_g0[32:36, :], eyeb[0:4, :], gx_cur[0:4, 1, :],
                                     start=True, stop=False, tile_position=(0, 32))
                    nc.tensor.matmul(ps_g0[96:100, :], eyeb[0:4, :], gx_cur[0:4, 2, :],
                                     start=True, stop=False, tile_position=(0, 96))
                    nc.tensor.matmul(ps_g0[64:68, :], ones_1_4[:], bhn0[:],
                                     start=True, stop=False, tile_position=(0, 64))
                    ps_g1 = g1bufs[t % 2]
                    nc.tensor.matmul(ps_g1[0:4, :], ones_1_4[:], brz1[:, 0:512],
                                     start=True, stop=False, tile_position=(0, 0))
                    nc.tensor.matmul(ps_g1[32:36, :], ones_1_4[:], brz1[:, 512:1024],
                                     start=True, stop=False, tile_position=(0, 32))
                    nc.tensor.matmul(ps_g1[64:68, :], ones_1_4[:], bhn1[:],
                                     start=True, stop=False, tile_position=(0, 64))
                    nc.tensor.matmul(ps_g1[96:100, :], ones_1_4[:], bin1[:],
                                     start=True, stop=False, tile_position=(0, 96))

                    # === PE: L0 h-part waves (r,z,hn col-tiled) ===
                    for kt in range(hkt):
                        lhs = h0T_prev[:, kt, :]
                        nc.tensor.matmul(ps_g0[0:4, :], lhs, w0h_rz[:, kt, 0:512],
                                         start=False, stop=False, tile_position=(0, 0))
                        nc.tensor.matmul(ps_g0[32:36, :], lhs, w0h_rz[:, kt, 512:1024],
                                         start=False, stop=False, tile_position=(0, 32))
                        nc.tensor.matmul(ps_g0[64:68, :], lhs, w0h_n[:, kt, :],
                                         start=False, stop=(kt == hkt - 1),
                                         tile_position=(0, 64))
                    # === PE: L1 h-part waves ===
                    for kt in range(hkt):
                        lhs = h1T_prev[:, kt, :]
                        nc.tensor.matmul(ps_g1[0:4, :], lhs, w1h_rz[:, kt, 0:512],
                                         start=False, stop=False, tile_position=(0, 0))
                        nc.tensor.matmul(ps_g1[32:36, :], lhs, w1h_rz[:, kt, 512:1024],
                                         start=False, stop=False, tile_position=(0, 32))
                        nc.tensor.matmul(ps_g1[64:68, :], lhs, w1h_n[:, kt, :],
                                         start=False, stop=(kt == hkt - 1),
                                         tile_position=(0, 64))

                    # === DVE/ACT: attention scores -> attexp (exp via sigmoid) ===
                    e1 = work.tile([128, st, BL], F32, tag="e1")
                    for j in range(st):
                        nc.vector.tensor_tensor(e1[:, j, :], maskT[:, j, :],
                                                ps_sbc, ALU.mult)
                    e2 = work.tile([128, st * BL], F32, tag="e2")
                    nc.vector.tensor_tensor(
                        e2[:], e1[:].rearrange("p a b -> p (a b)"),
                        enclT[:].rearrange("p a b -> p (a b)"), ALU.add)
                    e3 = work.tile([128, st * BL], F32, tag="e3")
                    nc.scalar.activation(e3[:], e2[:], AF.Relu)
                    sge = work.tile([128, st * BL], F32, tag="sge")
                    nc.scalar.activation(sge[:], e3[:], AF.Sigmoid)
                    den = work.tile([128, st * BL], F32, tag="den")
                    nc.vector.scalar_tensor_tensor(den[:], sge[:], -1.0, ones_f16[:],
                                                   ALU.mult, ALU.add)
                    rden = work.tile([128, st * BL], F32, tag="rden")
                    nc.vector.reciprocal(rden[:], den[:])
                    attexp = work.tile([128, st, BL], BF16, tag="attexp")
                    nc.vector.tensor_tensor(
                        attexp[:].rearrange("p a b -> p (a b)"), sge[:], rden[:],
                        ALU.mult)

                    # === sum(attexp) and 1/Z ===
                    ps_se = ps_small[0:st * BL, 8:9]
                    nc.tensor.matmul(ps_se,
                                     attexp[:].rearrange("p a b -> p (a b)"),
                                     ones_128_1[:], start=True, stop=True)
                    se_sb = work.tile([st * BL, 1], BF16, tag="se_sb")
                    nc.vector.tensor_copy(se_sb[:], ps_se)
                    ps_z = ps_small[0:1, 12:12 + BL]
                    nc.tensor.matmul(ps_z, se_sb[:], sel[:], start=True, stop=True)
                    zr = work.tile([1, BL], F32, tag="zr")
                    nc.vector.reciprocal(zr[:], ps_z)
                    zr_bf = work.tile([1, BL], BF16, tag="zr_bf")
                    nc.vector.tensor_copy(zr_bf[:], zr[:])
                    ps_zb = ps_small[:, 16:16 + BL]
                    nc.tensor.matmul(ps_zb, ones_1_128[:], zr_bf[:],
                                     start=True, stop=True)
                    attn = work.tile([128, st, BL], BF16, tag="attn")
                    for j in range(st):
                        nc.vector.tensor_tensor(attn[:, j, :], attexp[:, j, :],
                                                ps_zb, ALU.mult)

                    # === PE: c waves (col-tiled across b, spread rows) ===
                    for j in range(st):
                        for b in range(BL):
                            nc.tensor.matmul(
                                ps_c[32 * b:32 * b + 1, :], attn[:, j, b:b + 1],
                                hid[:, b, j, :], start=(j == 0), stop=(j == st - 1),
                                tile_position=(0, 32 * b))
                    c_sp = work.tile([128, 512], BF16, tag="c_sp")
                    nc.vector.tensor_copy(c_sp[:], ps_c[:])

                    # === PE: cT via 16 single-row transposes -> featsT c-part ===
                    ps_cT = ps_smb.tile([128, hkt, BL, 2], BF16, tag="cT")
                    for b in range(BL):
                        for hc in range(hkt):
                            nc.tensor.transpose(
                                ps_cT[:, hc, b, 0:1],
                                c_sp[32 * b:32 * b + 1, hc * 128:(hc + 1) * 128],
                                eyeb[32 * b:32 * b + 1, 0:1],
                                tile_position=(32 * b, 0))
                    nc.vector.tensor_copy(featsT[:, hkt:2 * hkt, tc0:tc1],
                                          ps_cT[:, :, :, 0])

                    # === PE: L0 c-part waves (r,z,in col-tiled) ===
                    for kt in range(hkt):
                        lhs = featsT[:, hkt + kt, tc0:tc1]
                        nc.tensor.matmul(ps_g0[0:4, :], lhs,
                                         w0i_rz[:, xkt + kt, 0:512],
                                         start=False, stop=(kt == hkt - 1),
                                         tile_position=(0, 0))
                        nc.tensor.matmul(ps_g0[32:36, :], lhs,
                                         w0i_rz[:, xkt + kt, 512:1024],
                                         start=False, stop=(kt == hkt - 1),
                                         tile_position=(0, 32))
                        nc.tensor.matmul(ps_g0[96:100, :], lhs,
                                         w0i_n[:, xkt + kt, :],
                                         start=False, stop=(kt == hkt - 1),
                                         tile_position=(0, 96))

                    # === L0 elementwise ===
                    sg0 = work.tile([36, 512], BF16, tag="sg0")
                    nc.scalar.activation(sg0[:], ps_g0[0:36, :], AF.Sigmoid)
                    m0 = work.tile([36, 512], BF16, tag="m0")
                    nc.vector.tensor_tensor(m0[32:36, :], sg0[0:4, :],
                                            ps_g0[64:68, :], ALU.mult)
                    np0 = work.tile([36, 512], BF16, tag="np0")
                    nc.vector.tensor_tensor(np0[32:36, :], m0[32:36, :],
                                            ps_g0[96:100, :], ALU.add)
                    n0 = work.tile([36, 512], BF16, tag="n0")
                    nc.scalar.activation(n0[32:36, :], np0[32:36, :], AF.Tanh)
                    w0 = work.tile([36, 512], F32, tag="w0")
                    nc.scalar.activation(w0[32:36, :], sg0[32:36, :], AF.Identity,
                                         bias=1.0, scale=-1.0)
                    u0 = work.tile([36, 512], F32, tag="u0")
                    nc.gpsimd.tensor_tensor(u0[32:36, :], sg0[32:36, :],
                                            h0n_prev[32:36, :], ALU.mult)
                    v0 = work.tile([36, 512], F32, tag="v0")
                    nc.vector.tensor_tensor(v0[32:36, :], n0[32:36, :], w0[32:36, :],
                                            ALU.mult)
                    h0n_new = hpool.tile([36, H], F32, tag="h0n")
                    nc.vector.tensor_tensor(h0n_new[32:36, :], v0[32:36, :],
                                            u0[32:36, :], ALU.add)

                    # === PE: h0T transposes ===
                    ps_h0T = ps_small[:, 32:32 + hkt * BL].rearrange(
                        "p (a b) -> p a b", a=hkt)
                    for hc in range(hkt):
                        nc.tensor.transpose(
                            ps_h0T[:, hc, :],
                            h0n_new[32:36, hc * 128:(hc + 1) * 128],
                            eyef[32:36, :], tile_position=(32, 0))
                    h0T_sb = hpool.tile([128, hkt, BL], BF16, tag="h0T")
                    nc.vector.tensor_copy(h0T_sb[:], ps_h0T)

                    # === PE: L1 i-part waves (from h0T) ===
                    for kt in range(hkt):
                        lhs = h0T_sb[:, kt, :]
                        nc.tensor.matmul(ps_g1[0:4, :], lhs, w1i_rz[:, kt, 0:512],
                                         start=False, stop=(kt == hkt - 1),
                                         tile_position=(0, 0))
                        nc.tensor.matmul(ps_g1[32:36, :], lhs,
                                         w1i_rz[:, kt, 512:1024],
                                         start=False, stop=(kt == hkt - 1),
                                         tile_position=(0, 32))
                        nc.tensor.matmul(ps_g1[96:100, :], lhs, w1i_n[:, kt, :],
                                         start=False, stop=(kt == hkt - 1),
                                         tile_position=(0, 96))

                    # === L1 elementwise ===
                    sg1 = work.tile([36, 512], BF16, tag="sg1")
                    nc.scalar.activation(sg1[:], ps_g1[0:36, :], AF.Sigmoid)
                    m1 = work.tile([36, 512], BF16, tag="m1")
                    nc.vector.tensor_tensor(m1[32:36, :], sg1[0:4, :],
                                            ps_g1[64:68, :], ALU.mult)
                    np1 = work.tile([36, 512], BF16, tag="np1")
                    nc.vector.tensor_tensor(np1[32:36, :], m1[32:36, :],
                                            ps_g1[96:100, :], ALU.add)
                    n1 = work.tile([36, 512], BF16, tag="n1")
                    nc.scalar.activation(n1[32:36, :], np1[32:36, :], AF.Tanh)
                    w1 = work.tile([36, 512], F32, tag="w1")
                    nc.scalar.activation(w1[32:36, :], sg1[32:36, :], AF.Identity,
                                         bias=1.0, scale=-1.0)
                    u1 = work.tile([36, 512], F32, tag="u1")
                    nc.gpsimd.tensor_tensor(u1[32:36, :], sg1[32:36, :],
                                            h1n_prev[32:36, :], ALU.mult)
                    v1 = work.tile([36, 512], F32, tag="v1")
                    nc.vector.tensor_tensor(v1[32:36, :], n1[32:36, :], w1[32:36, :],
                                            ALU.mult)
                    h1n_new = hpool.tile([36, H], F32, tag="h1n")
                    nc.vector.tensor_tensor(h1n_new[32:36, :], v1[32:36, :],
                                            u1[32:36, :], ALU.add)

                    # === PE: h1T transposes -> featsT h-part ===
                    ps_h1T = ps_small[:, 48:48 + hkt * BL].rearrange(
                        "p (a b) -> p a b", a=hkt)
                    for hc in range(hkt):
                        nc.tensor.transpose(
                            ps_h1T[:, hc, :],
                            h1n_new[32:36, hc * 128:(hc + 1) * 128],
                            eyef[32:36, :], tile_position=(32, 0))
                    nc.vector.tensor_copy(featsT[:, 0:hkt, tc0:tc1], ps_h1T)

                    h0T_prev = h0T_sb[:, :, :]
                    h0n_prev, h1n_prev = h0n_new, h1n_new

            # ---- phase B: projection + log-softmax ----
            with ExitStack() as bctx:
                bsing = bctx.enter_context(tc.tile_pool(name="bsing", bufs=1))
                wstr = bctx.enter_context(tc.tile_pool(name="wstr", bufs=3))
                escp = bctx.enter_context(tc.tile_pool(name="escp", bufs=2))
                otp = bctx.enter_context(tc.tile_pool(name="otp", bufs=6))
                stat = bctx.enter_context(tc.tile_pool(name="stat", bufs=1))
                ps_b = bctx.enter_context(tc.tile_pool(name="ps_b", bufs=4,
                                                       space="PSUM"))
                kt2 = 2 * hkt
                logits = bsing.tile([128, mch, nvt, VT], BF16)
                sums = bsing.tile([128, mch, nvt], F32)
                feats8 = bsing.tile([128, kt2, cfg.rows], FP8)
                nc.scalar.mul(feats8[:], featsT[:], SF)
                sinv = 1.0 / (SW * SF)

                for v in range(nvt):
                    wt = wstr.tile([128, kt2, 512], FP8, tag="wt")
                    nc.vector.memset(wt[:, :, VT:512], 0.0)
                    nc.sync.dma_start(wt[:, :, 0:VT],
                                      d_outWT[:, :, v * VT:(v + 1) * VT])
                    bt = wstr.tile([1, VT], BF16, tag="bt")
                    nc.sync.dma_start(bt[:], d_outb[:, v * VT:(v + 1) * VT])
                    for m in range(mch):
                        acc = ps_b.tile([128, 512], F32, tag="acc")
                        for kt in range(kt2 // 2):
                            nc.tensor.matmul(
                                acc[:],
                                feats8[:, 2 * kt:2 * kt + 2, m * 128:(m + 1) * 128],
                                wt[:, 2 * kt:2 * kt + 2, :],
                                start=(kt == 0), stop=False,
                                perf_mode=mybir.MatmulPerfMode.DoubleRow)
                        nc.tensor.matmul(acc[:, 0:VT], ones_1_128[:], bt[:],
                                         start=False, stop=True)
                        nc.vector.tensor_scalar_mul(logits[:, m, v, :],
                                                    acc[:, 0:VT], sinv)
                        esc = escp.tile([128, VT], BF16, tag="esc")
                        nc.scalar.activation(esc[:], acc[:, 0:VT], AF.Exp,
                                             scale=sinv,
                                             accum_out=sums[:, m, v:v + 1])

                nlz = stat.tile([128, mch], F32)
                for m in range(mch):
                    gsum = stat.tile([128, 1], F32, tag=f"gs{m}")
                    nc.vector.tensor_reduce(gsum[:], sums[:, m, :],
                                            mybir.AxisListType.X, ALU.add)
                    lnz = stat.tile([128, 1], F32, tag=f"ln{m}")
                    nc.scalar.activation(lnz[:], gsum[:], AF.Ln)
                    nc.vector.tensor_scalar_mul(nlz[:, m:m + 1], lnz[:], -1.0)

                for v in range(nvt):
                    for m in range(mch):
                        k = (v * mch + m) % 3
                        ot = otp.tile([128, VT], F32, tag=f"ot{k}")
                        if k == 0:
                            nc.vector.tensor_scalar_add(ot[:], logits[:, m, v, :],
                                                        nlz[:, m:m + 1])
                        elif k == 1:
                            nc.scalar.activation(ot[:], logits[:, m, v, :],
                                                 AF.Identity, bias=nlz[:, m:m + 1])
                        else:
                            nc.gpsimd.tensor_scalar_add(ot[:], logits[:, m, v, :],
                                                        nlz[:, m:m + 1])
                        nc.sync.dma_start(
                            d_out[m * 128:(m + 1) * 128, v * VT:(v + 1) * VT],
                            ot[:])
        for _rep in range(reps):
            one_rep()
    return nc


# ----------------------------------------------------------------------------
# host-side prep
# ----------------------------------------------------------------------------

def _prep_core(cfg, inputs, lengths, final_hidden, hiddens, att_W, att_b,
               wdict, core):
    T, S, H, IN, V = cfg.T, cfg.S, cfg.H, cfg.IN, cfg.V
    st, hkt, xkt = cfg.st, cfg.hkt, cfg.xkt
    bs = slice(core * BL, (core + 1) * BL)
    bf = ml_dtypes.bfloat16

    hid_c = hiddens[bs]                                   # (BL, S, H)
    hid_l = np.ascontiguousarray(
        hid_c.reshape(BL, st, 128, H).transpose(2, 0, 1, 3)).astype(bf)
    x_c = inputs[bs]                                      # (BL, T, IN)
    xT = np.ascontiguousarray(
        x_c.transpose(2, 1, 0).reshape(xkt, 128, T * BL).transpose(1, 0, 2)
    ).astype(bf)
    mask = (np.arange(S)[None, :] < np.asarray(lengths)[bs, None]).astype(
        np.float32)
    aW_h = att_W[0, :H].astype(np.float32)
    aW_e = att_W[0, H:].astype(np.float32)
    encl = hid_c.astype(np.float32) @ aW_e + float(att_b[0])
    maskT = np.ascontiguousarray(
        mask.T.reshape(st, 128, BL).transpose(1, 0, 2)).astype(np.float32)
    enclT = np.ascontiguousarray(
        encl.T.reshape(st, 128, BL).transpose(1, 0, 2)).astype(np.float32)
    aWhT = aW_h.reshape(hkt, 128, 1).transpose(1, 0, 2).astype(bf)
    hn = final_hidden[:, bs, :].astype(np.float32)        # (2, BL, H)
    hT0 = np.zeros((128, 2 * hkt, BL), np.float32)
    for layer in range(2):
        hT0[:, layer * hkt:(layer + 1) * hkt, :] = (
            hn[layer].T.reshape(hkt, 128, BL).transpose(1, 0, 2))
    eyeb = np.zeros((128, BL), np.float32)
    for a in range(4):
        eyeb[32 * a:32 * a + BL, :] = np.eye(BL)
    in_map = dict(
        hid=np.ascontiguousarray(hid_l),
        xT=xT,
        maskT=maskT,
        enclT=enclT,
        aWhT=np.ascontiguousarray(aWhT),
        hT0=hT0.astype(bf),
        hnat0=hn,
        sel=np.kron(np.ones((st, 1), np.float32),
                    np.eye(BL, dtype=np.float32)).astype(bf),
        eyeb=eyeb.astype(bf),
        eyef=eyeb,
    )
    in_map.update(wdict)
    return in_map


def _prep_shared(cfg, att_W, out_W, out_b, W_ih0, W_hh0, b_ih0, b_hh0,
                 W_ih1, W_hh1, b_ih1, b_hh1):
    H, V = cfg.H, cfg.V
    hkt, xkt = cfg.hkt, cfg.xkt
    bf = ml_dtypes.bfloat16
    G2 = 2 * H

    def kt_layout(Wt, nkt):
        K, N = Wt.shape
        assert K == nkt * 128
        return np.ascontiguousarray(
            Wt.reshape(nkt, 128, N).transpose(1, 0, 2)).astype(bf)

    w = {}
    w["w0i_rz"] = kt_layout(W_ih0[:G2, :].T.astype(np.float32), xkt + hkt)
    w["w0i_n"] = kt_layout(W_ih0[G2:, :].T.astype(np.float32), xkt + hkt)
    w["w0h_rz"] = kt_layout(W_hh0[:G2, :].T.astype(np.float32), hkt)
    w["w0h_n"] = kt_layout(W_hh0[G2:, :].T.astype(np.float32), hkt)
    w["w1i_rz"] = kt_layout(W_ih1[:G2, :].T.astype(np.float32), hkt)
    w["w1i_n"] = kt_layout(W_ih1[G2:, :].T.astype(np.float32), hkt)
    w["w1h_rz"] = kt_layout(W_hh1[:G2, :].T.astype(np.float32), hkt)
    w["w1h_n"] = kt_layout(W_hh1[G2:, :].T.astype(np.float32), hkt)
    w["brz0"] = (b_ih0[:G2] + b_hh0[:G2]).reshape(1, G2).astype(bf)
    w["bin0"] = b_ih0[G2:].reshape(1, H).astype(bf)
    w["bhn0"] = b_hh0[G2:].reshape(1, H).astype(bf)
    w["brz1"] = (b_ih1[:G2] + b_hh1[:G2]).reshape(1, G2).astype(bf)
    w["bin1"] = b_ih1[G2:].reshape(1, H).astype(bf)
    w["bhn1"] = b_hh1[G2:].reshape(1, H).astype(bf)
    wf = out_W.T.astype(np.float32) * 256.0
    w["outWT"] = np.ascontiguousarray(
        wf.reshape(2 * hkt, 128, V).transpose(1, 0, 2)).astype(
        ml_dtypes.float8_e4m3fn)
    w["outb"] = (out_b * (256.0 * 16.0)).reshape(1, V).astype(bf)
    return w


_CACHED = {}


def _collect(inputs):
    g = lambda k: np.asarray(inputs[k], dtype=np.float32)
    return dict(
        inp=g("inputs"), lengths=np.asarray(inputs["lengths"]),
        final_hidden=g("final_hidden"), hiddens=g("hiddens"),
        att_W=g("att_W"), att_b=g("att_b"), out_W=g("out_W"), out_b=g("out_b"),
        W_ih0=g("W_ih0"), W_hh0=g("W_hh0"), b_ih0=g("b_ih0"), b_hh0=g("b_hh0"),
        W_ih1=g("W_ih1"), W_hh1=g("W_hh1"), b_ih1=g("b_ih1"), b_hh1=g("b_hh1"))


def _make_in_maps(cfg, a):
    wdict = _prep_shared(cfg, a["att_W"], a["out_W"], a["out_b"],
                         a["W_ih0"], a["W_hh0"], a["b_ih0"], a["b_hh0"],
                         a["W_ih1"], a["W_hh1"], a["b_ih1"], a["b_hh1"])
    return [
        _prep_core(cfg, a["inp"], a["lengths"], a["final_hidden"],
                   a["hiddens"], a["att_W"], a["att_b"], wdict, core)
        for core in range(NCORES)
    ]


def build_for_bench(**inputs):
    import os
    cfg = Cfg()
    a = _collect(inputs)
    nc = build(cfg, reps=int(os.environ.get("KREPS", "1")))
    if not nc.is_finalized():
        nc.finalize()
    return nc, _make_in_maps(cfg, a)


def assemble_output(results, inputs):
    cfg = Cfg()
    outs = []
    for c in range(NCORES):
        o = results[c]["out"].reshape(cfg.T, BL, cfg.V).transpose(1, 0, 2)
        outs.append(o)
    return np.concatenate(outs, axis=0).astype(np.float32)


def kernel(**inputs):
    cfg = Cfg()
    a = _collect({k: np.asarray(v) if not np.isscalar(v) else v
                  for k, v in inputs.items()})
    from concourse.bass_utils import run_bass_kernel_spmd
    if "nc" not in _CACHED:
        nc = build(cfg)
        if not nc.is_finalized():
            nc.finalize()
        _CACHED["nc"] = nc
    nc = _CACHED["nc"]
    in_maps = _make_in_maps(cfg, a)
    res = run_bass_kernel_spmd(nc, in_maps, list(range(NCORES)))
    return assemble_output(res.results, inputs)



# revision 32
# speedup vs baseline: 1.4479x; 1.4479x over previous
"""Trainium2 Bass kernel v2 for nn_Decoder — col-tiled recurrence.

Key differences vs baseline:
- Gate matmuls col-tiled across PSUM strips (r@0,z@32,hn@64,in@96) for PE
  concurrency; biases/x-parts injected into PSUM via tiny I4/ones matmuls.
- No DRAM round-trip transposes: PE is_transpose for c/h0/h1 (on-chip).
- Single activation table (sigmoid set) for the whole recurrence: exp(x)
  computed as sigmoid(x)/(1-sigmoid(x)); phase B uses the exp/ln set once.
- Phase B: weight tile streamed once (v-outer), exp-accum online, per-row
  log-sum-exp without max subtraction, multi-engine normalize tail.
"""

import sys

sys.path.insert(0, "/opt/trn_rl_repo")

import numpy as np
import ml_dtypes

import concourse.bass as bass
import concourse.bacc as bacc
import concourse.tile as tile
from concourse import mybir
from contextlib import ExitStack

F32 = mybir.dt.float32
BF16 = mybir.dt.bfloat16
FP8 = mybir.dt.float8e4
SW, SF = 256.0, 16.0  # fp8 scales for out_W / feats
AF = mybir.ActivationFunctionType
ALU = mybir.AluOpType

BL = 4
NCORES = 8


class Cfg:
    def __init__(self, T=64, S=512, H=512, IN=256, V=32000, VT=500):
        self.T, self.S, self.H, self.IN, self.V, self.VT = T, S, H, IN, V, VT
        self.st = S // 128
        self.hkt = H // 128
        self.xkt = IN // 128
        self.nvt = V // VT
        self.rows = T * BL
        self.mch = self.rows // 128
        assert S % 128 == 0 and H % 128 == 0 and IN % 128 == 0
        assert V % VT == 0 and VT <= 512 and self.rows % 128 == 0


def build(cfg: Cfg, reps: int = 1):
    T, S, H, IN, V, VT = cfg.T, cfg.S, cfg.H, cfg.IN, cfg.V, cfg.VT
    st, hkt, xkt, nvt, mch = cfg.st, cfg.hkt, cfg.xkt, cfg.nvt, cfg.mch
    G2 = 2 * H

    nc = bacc.Bacc()

    # ---- DRAM I/O ----
    d_hid = nc.dram_tensor("hid", [128, BL, st, H], BF16, kind="ExternalInput")
    d_xT = nc.dram_tensor("xT", [128, xkt, cfg.rows], BF16, kind="ExternalInput")
    d_maskT = nc.dram_tensor("maskT", [128, st, BL], F32, kind="ExternalInput")
    d_enclT = nc.dram_tensor("enclT", [128, st, BL], F32, kind="ExternalInput")
    d_aWhT = nc.dram_tensor("aWhT", [128, hkt, 1], BF16, kind="ExternalInput")
    d_hT0 = nc.dram_tensor("hT0", [128, 2 * hkt, BL], BF16, kind="ExternalInput")
    d_hnat0 = nc.dram_tensor("hnat0", [2, BL, H], F32, kind="ExternalInput")
    d_w0i_rz = nc.dram_tensor("w0i_rz", [128, xkt + hkt, G2], BF16, kind="ExternalInput")
    d_w0i_n = nc.dram_tensor("w0i_n", [128, xkt + hkt, H], BF16, kind="ExternalInput")
    d_w0h_rz = nc.dram_tensor("w0h_rz", [128, hkt, G2], BF16, kind="ExternalInput")
    d_w0h_n = nc.dram_tensor("w0h_n", [128, hkt, H], BF16, kind="ExternalInput")
    d_w1i_rz = nc.dram_tensor("w1i_rz", [128, hkt, G2], BF16, kind="ExternalInput")
    d_w1i_n = nc.dram_tensor("w1i_n", [128, hkt, H], BF16, kind="ExternalInput")
    d_w1h_rz = nc.dram_tensor("w1h_rz", [128, hkt, G2], BF16, kind="ExternalInput")
    d_w1h_n = nc.dram_tensor("w1h_n", [128, hkt, H], BF16, kind="ExternalInput")
    d_brz0 = nc.dram_tensor("brz0", [1, G2], BF16, kind="ExternalInput")
    d_bin0 = nc.dram_tensor("bin0", [1, H], BF16, kind="ExternalInput")
    d_bhn0 = nc.dram_tensor("bhn0", [1, H], BF16, kind="ExternalInput")
    d_brz1 = nc.dram_tensor("brz1", [1, G2], BF16, kind="ExternalInput")
    d_bin1 = nc.dram_tensor("bin1", [1, H], BF16, kind="ExternalInput")
    d_bhn1 = nc.dram_tensor("bhn1", [1, H], BF16, kind="ExternalInput")
    d_sel = nc.dram_tensor("sel", [st * BL, BL], BF16, kind="ExternalInput")
    d_eyeb = nc.dram_tensor("eyeb", [128, BL], BF16, kind="ExternalInput")
    d_eyef = nc.dram_tensor("eyef", [128, BL], F32, kind="ExternalInput")
    d_outWT = nc.dram_tensor("outWT", [128, 2 * hkt, V], FP8, kind="ExternalInput")
    d_outb = nc.dram_tensor("outb", [1, V], BF16, kind="ExternalInput")
    d_out = nc.dram_tensor("out", [cfg.rows, V], BF16, kind="ExternalOutput")

    with tile.TileContext(nc) as tc, ExitStack() as octx:
        keep = octx.enter_context(tc.tile_pool(name="keep", bufs=1))
        featsT = keep.tile([128, 2 * hkt, cfg.rows], BF16)
        ones_1_128 = keep.tile([1, 128], BF16)
        nc.vector.memset(ones_1_128[:], 1.0)

        def one_rep():
            with ExitStack() as actx:
                sing = actx.enter_context(tc.tile_pool(name="sing", bufs=1))
                work = actx.enter_context(tc.tile_pool(name="work", bufs=2))
                hpool = actx.enter_context(tc.tile_pool(name="hpool", bufs=2))
                gxp = actx.enter_context(tc.tile_pool(name="gxp", bufs=2))
                dpool = actx.enter_context(tc.tile_pool(name="dram", bufs=1, space="DRAM"))
                ps_g0p = actx.enter_context(tc.tile_pool(name="ps_g0", bufs=1, space="PSUM"))
                ps_g1p = actx.enter_context(tc.tile_pool(name="ps_g1", bufs=1, space="PSUM"))
                ps_cp = actx.enter_context(tc.tile_pool(name="ps_c", bufs=1, space="PSUM"))
                ps_smf = actx.enter_context(tc.tile_pool(name="ps_smf", bufs=2, space="PSUM"))
                ps_smb = actx.enter_context(tc.tile_pool(name="ps_smb", bufs=1, space="PSUM"))

                # ---- resident loads ----
                hid = sing.tile([128, BL, st, H], BF16)
                nc.sync.dma_start(hid[:], d_hid[:])
                xT = sing.tile([128, xkt, cfg.rows], BF16)
                nc.sync.dma_start(xT[:], d_xT[:])
                maskT = sing.tile([128, st, BL], F32)
                nc.sync.dma_start(maskT[:], d_maskT[:])
                enclT = sing.tile([128, st, BL], F32)
                nc.sync.dma_start(enclT[:], d_enclT[:])
                aWhT = sing.tile([128, hkt, 1], BF16)
                nc.sync.dma_start(aWhT[:], d_aWhT[:])
                hT_init = sing.tile([128, 2 * hkt, BL], BF16)
                nc.sync.dma_start(hT_init[:], d_hT0[:])
                w0i_rz = sing.tile([128, xkt + hkt, G2], BF16)
                nc.sync.dma_start(w0i_rz[:], d_w0i_rz[:])
                w0i_n = sing.tile([128, xkt + hkt, H], BF16)
                nc.sync.dma_start(w0i_n[:], d_w0i_n[:])
                w0h_rz = sing.tile([128, hkt, G2], BF16)
                nc.sync.dma_start(w0h_rz[:], d_w0h_rz[:])
                w0h_n = sing.tile([128, hkt, H], BF16)
                nc.sync.dma_start(w0h_n[:], d_w0h_n[:])
                w1i_rz = sing.tile([128, hkt, G2], BF16)
                nc.sync.dma_start(w1i_rz[:], d_w1i_rz[:])
                w1i_n = sing.tile([128, hkt, H], BF16)
                nc.sync.dma_start(w1i_n[:], d_w1i_n[:])
                w1h_rz = sing.tile([128, hkt, G2], BF16)
                nc.sync.dma_start(w1h_rz[:], d_w1h_rz[:])
                w1h_n = sing.tile([128, hkt, H], BF16)
                nc.sync.dma_start(w1h_n[:], d_w1h_n[:])
                brz0 = sing.tile([1, G2], BF16)
                nc.sync.dma_start(brz0[:], d_brz0[:])
                bin0 = sing.tile([1, H], BF16)
                nc.sync.dma_start(bin0[:], d_bin0[:])
                bhn0 = sing.tile([1, H], BF16)
                nc.sync.dma_start(bhn0[:], d_bhn0[:])
                brz1 = sing.tile([1, G2], BF16)
                nc.sync.dma_start(brz1[:], d_brz1[:])
                bin1 = sing.tile([1, H], BF16)
                nc.sync.dma_start(bin1[:], d_bin1[:])
                bhn1 = sing.tile([1, H], BF16)
                nc.sync.dma_start(bhn1[:], d_bhn1[:])
                sel = sing.tile([st * BL, BL], BF16)
                nc.sync.dma_start(sel[:], d_sel[:])
                eyeb = sing.tile([128, BL], BF16)
                nc.sync.dma_start(eyeb[:], d_eyeb[:])
                eyef = sing.tile([128, BL], F32)
                nc.sync.dma_start(eyef[:], d_eyef[:])

                ones_1_4 = sing.tile([1, BL], BF16)
                nc.vector.memset(ones_1_4[:], 1.0)
                ones_128_1 = sing.tile([128, 1], BF16)
                nc.vector.memset(ones_128_1[:], 1.0)
                ones_f16 = sing.tile([128, st * BL], F32)
                nc.vector.memset(ones_f16[:], 1.0)
                ones_h = sing.tile([36, 512], F32)
                nc.vector.memset(ones_h[:], 1.0)

                h0nat_init = sing.tile([36, H], F32)
                nc.sync.dma_start(h0nat_init[32:36, :], d_hnat0[0, :, :])
                h1nat_init = sing.tile([36, H], F32)
                nc.sync.dma_start(h1nat_init[32:36, :], d_hnat0[1, :, :])

                # ---- hoisted psum tiles (ping-pong) ----
                g0a = ps_g0p.tile([128, 512], F32, tag="g0a", name="g0a")
                g0b = ps_g0p.tile([128, 512], F32, tag="g0b", name="g0b")
                g1a = ps_g1p.tile([128, 512], F32, tag="g1a", name="g1a")
                g1b = ps_g1p.tile([128, 512], F32, tag="g1b", name="g1b")
                g0bufs, g1bufs = [g0a, g0b], [g1a, g1b]
                ps_c = ps_cp.tile([128, 512], F32, tag="c")
                for tl in (g0a, g0b, g1a, g1b, ps_c):
                    nc.vector.memset(tl[:], 0.0)

                # ---- gi0x precompute: x-part of L0 gates (+brz0/bin0 biases) ----
                # DRAM layout [rows, 3, 512]; chunk n: 0=r, 1=z, 2=n-gate
                gi0x_dram = dpool.tile([cfg.rows, 3, 512], BF16, tag="gi0x")
                for m in range(mch):
                    for n in range(3):
                        acc = g0bufs[(m * 3 + n) % 2]
                        for kt in range(xkt):
                            rhs = (w0i_rz[:, kt, n * 512:(n + 1) * 512] if n < 2
                                   else w0i_n[:, kt, :])
                            nc.tensor.matmul(
                                acc[:], xT[:, kt, m * 128:(m + 1) * 128], rhs,
                                start=(kt == 0), stop=False)
                        brow = brz0[:, n * 512:(n + 1) * 512] if n < 2 else bin0[:]
                        nc.tensor.matmul(acc[:], ones_1_128[:], brow,
                                         start=False, stop=True)
                        gxs = work.tile([128, 512], BF16, tag="gxs", bufs=2)
                        nc.vector.tensor_copy(gxs[:], acc[:])
                        nc.sync.dma_start(
                            gi0x_dram[m * 128:(m + 1) * 128, n, :], gxs[:])

                # ---- recurrence ----
                h0T_prev = hT_init[:, 0:hkt, :]
                h0n_prev = h0nat_init
                h1n_prev = h1nat_init

                gx_t = gxp.tile([BL, 3, 512], BF16, tag="gx")
                nc.sync.dma_start(gx_t[:], gi0x_dram[0:BL, :, :])

                for t in range(T):
                    tc0, tc1 = t * BL, (t + 1) * BL
                    h1T_prev = (hT_init[:, hkt:2 * hkt, :] if t == 0
                                else featsT[:, 0:hkt, (t - 1) * BL:t * BL])

                    # prefetch next step's gx
                    gx_cur = gx_t
                    if t + 1 < T:
                        gx_t = gxp.tile([BL, 3, 512], BF16, tag="gx")
                        nc.sync.dma_start(gx_t[:],
                                          gi0x_dram[(t + 1) * BL:(t + 2) * BL, :, :])

                    # === PE: s = aWh . h1_prev -> [1, BL] ===
                    ps_small = ps_smf.tile([128, 64], F32, tag="sm")
                    ps_s = ps_small[0:1, 0:BL]
                    for kt in range(hkt):
                        nc.tensor.matmul(ps_s[:], aWhT[:, kt, :], h1T_prev[:, kt, :],
                                         start=(kt == 0), stop=(kt == hkt - 1))
                    s_sb = work.tile([1, BL], BF16, tag="s_sb")
                    nc.vector.tensor_copy(s_sb[:], ps_s[:])
                    ps_sbc = ps_small[:, 4:4 + BL]
                    nc.tensor.matmul(ps_sbc, ones_1_128[:], s_sb[:],
                                     start=True, stop=True)

                    # === PE: inject gx (x-part+biases) and biases into gate psums
                    ps_g0 = g0bufs[t % 2]
                    nc.tensor.matmul(ps_g0[0:4, :], eyeb[0:4, :], gx_cur[0:4, 0, :],
                                     start=True, stop=False, tile_position=(0, 0))
                    nc.tensor.matmul(ps_g0[32:36, :], eyeb[0:4, :], gx_cur[0:4, 1, :],
                                     start=True, stop=False, tile_position=(0, 32))
                    nc.tensor.matmul(ps_g0[96:100, :], eyeb[0:4, :], gx_cur[0:4, 2, :],
                                     start=True, stop=False, tile_position=(0, 96))
                    nc.tensor.matmul(ps_g0[64:68, :], ones_1_4[:], bhn0[:],
                                     start=True, stop=False, tile_position=(0, 64))
                    ps_g1 = g1bufs[t % 2]
                    nc.tensor.matmul(ps_g1[0:4, :], ones_1_4[:], brz1[:, 0:512],
                                     start=True, stop=False, tile_position=(0, 0))
                    nc.tensor.matmul(ps_g1[32:36, :], ones_1_4[:], brz1[:, 512:1024],
                                     start=True, stop=False, tile_position=(0, 32))
                    nc.tensor.matmul(ps_g1[64:68, :], ones_1_4[:], bhn1[:],
                                     start=True, stop=False, tile_position=(0, 64))
                    nc.tensor.matmul(ps_g1[96:100, :], ones_1_4[:], bin1[:],
                                     start=True, stop=False, tile_position=(0, 96))

                    # === PE: L0 h-part waves (r,z,hn col-tiled) ===
                    for kt in range(hkt):
                        lhs = h0T_prev[:, kt, :]
                        nc.tensor.matmul(ps_g0[0:4, :], lhs, w0h_rz[:, kt, 0:512],
                                         start=False, stop=False, tile_position=(0, 0))
                        nc.tensor.matmul(ps_g0[32:36, :], lhs, w0h_rz[:, kt, 512:1024],
                                         start=False, stop=False, tile_position=(0, 32))
                        nc.tensor.matmul(ps_g0[64:68, :], lhs, w0h_n[:, kt, :],
                                         start=False, stop=(kt == hkt - 1),
                                         tile_position=(0, 64))
                    # === PE: L1 h-part waves ===
                    for kt in range(hkt):
                        lhs = h1T_prev[:, kt, :]
                        nc.tensor.matmul(ps_g1[0:4, :], lhs, w1h_rz[:, kt, 0:512],
                                         start=False, stop=False, tile_position=(0, 0))
                        nc.tensor.matmul(ps_g1[32:36, :], lhs, w1h_rz[:, kt, 512:1024],
                                         start=False, stop=False, tile_position=(0, 32))
                        nc.tensor.matmul(ps_g1[64:68, :], lhs, w1h_n[:, kt, :],
                                         start=False, stop=(kt == hkt - 1),
                                         tile_position=(0, 64))

                    # === DVE/ACT: attention scores -> attexp (exp via sigmoid) ===
                    e1 = work.tile([128, st, BL], F32, tag="e1")
                    for j in range(st):
                        nc.vector.tensor_tensor(e1[:, j, :], maskT[:, j, :],
                                                ps_sbc, ALU.mult)
                    e2 = work.tile([128, st * BL], F32, tag="e2")
                    nc.vector.tensor_tensor(
                        e2[:], e1[:].rearrange("p a b -> p (a b)"),
                        enclT[:].rearrange("p a b -> p (a b)"), ALU.add)
                    e3 = work.tile([128, st * BL], F32, tag="e3")
                    nc.scalar.activation(e3[:], e2[:], AF.Relu)
                    sge = work.tile([128, st * BL], F32, tag="sge")
                    nc.scalar.activation(sge[:], e3[:], AF.Sigmoid)
                    den = work.tile([128, st * BL], F32, tag="den")
                    nc.vector.scalar_tensor_tensor(den[:], sge[:], -1.0, ones_f16[:],
                                                   ALU.mult, ALU.add)
                    rden = work.tile([128, st * BL], F32, tag="rden")
                    nc.vector.reciprocal(rden[:], den[:])
                    attexp = work.tile([128, st, BL], BF16, tag="attexp")
                    nc.vector.tensor_tensor(
                        attexp[:].rearrange("p a b -> p (a b)"), sge[:], rden[:],
                        ALU.mult)

                    # === sum(attexp) and 1/Z ===
                    ps_se = ps_small[0:st * BL, 8:9]
                    nc.tensor.matmul(ps_se,
                                     attexp[:].rearrange("p a b -> p (a b)"),
                                     ones_128_1[:], start=True, stop=True)
                    se_sb = work.tile([st * BL, 1], BF16, tag="se_sb")
                    nc.vector.tensor_copy(se_sb[:], ps_se)
                    ps_z = ps_small[0:1, 12:12 + BL]
                    nc.tensor.matmul(ps_z, se_sb[:], sel[:], start=True, stop=True)
                    zr = work.tile([1, BL], F32, tag="zr")
                    nc.vector.reciprocal(zr[:], ps_z)
                    zr_bf = work.tile([1, BL], BF16, tag="zr_bf")
                    nc.vector.tensor_copy(zr_bf[:], zr[:])
                    ps_zb = ps_small[:, 16:16 + BL]
                    nc.tensor.matmul(ps_zb, ones_1_128[:], zr_bf[:],
                                     start=True, stop=True)
                    attn = work.tile([128, st, BL], BF16, tag="attn")
                    for j in range(st):
                        nc.vector.tensor_tensor(attn[:, j, :], attexp[:, j, :],
                                                ps_zb, ALU.mult)

                    # === PE: c waves (col-tiled across b, spread rows) ===
                    for j in range(st):
                        for b in range(BL):
                            nc.tensor.matmul(
                                ps_c[32 * b:32 * b + 1, :], attn[:, j, b:b + 1],
                                hid[:, b, j, :], start=(j == 0), stop=(j == st - 1),
                                tile_position=(0, 32 * b))
                    c_sp = work.tile([128, 512], BF16, tag="c_sp")
                    nc.vector.tensor_copy(c_sp[:], ps_c[:])

                    # === PE: cT via 16 single-row transposes -> featsT c-part ===
                    ps_cT = ps_smb.tile([128, hkt, BL, 2], BF16, tag="cT")
                    for b in range(BL):
                        for hc in range(hkt):
                            nc.tensor.transpose(
                                ps_cT[:, hc, b, 0:1],
                                c_sp[32 * b:32 * b + 1, hc * 128:(hc + 1) * 128],
                                eyeb[32 * b:32 * b + 1, 0:1],
                                tile_position=(32 * b, 0))
                    nc.vector.tensor_copy(featsT[:, hkt:2 * hkt, tc0:tc1],
                                          ps_cT[:, :, :, 0])

                    # === PE: L0 c-part waves (r,z,in col-tiled) ===
                    for kt in range(hkt):
                        lhs = featsT[:, hkt + kt, tc0:tc1]
                        nc.tensor.matmul(ps_g0[0:4, :], lhs,
                                         w0i_rz[:, xkt + kt, 0:512],
                                         start=False, stop=(kt == hkt - 1),
                                         tile_position=(0, 0))
                        nc.tensor.matmul(ps_g0[32:36, :], lhs,
                                         w0i_rz[:, xkt + kt, 512:1024],
                                         start=False, stop=(kt == hkt - 1),
                                         tile_position=(0, 32))
                        nc.tensor.matmul(ps_g0[96:100, :], lhs,
                                         w0i_n[:, xkt + kt, :],
                                         start=False, stop=(kt == hkt - 1),
                                         tile_position=(0, 96))

                    # === L0 elementwise ===
                    sg0 = work.tile([36, 512], BF16, tag="sg0")
                    nc.scalar.activation(sg0[:], ps_g0[0:36, :], AF.Sigmoid)
                    m0 = work.tile([36, 512], BF16, tag="m0")
                    nc.vector.tensor_tensor(m0[32:36, :], sg0[0:4, :],
                                            ps_g0[64:68, :], ALU.mult)
                    np0 = work.tile([36, 512], BF16, tag="np0")
                    nc.vector.tensor_tensor(np0[32:36, :], m0[32:36, :],
                                            ps_g0[96:100, :], ALU.add)
                    n0 = work.tile([36, 512], BF16, tag="n0")
                    nc.scalar.activation(n0[32:36, :], np0[32:36, :], AF.Tanh)
                    w0 = work.tile([36, 512], F32, tag="w0")
                    nc.scalar.activation(w0[32:36, :], sg0[32:36, :], AF.Identity,
                                         bias=1.0, scale=-1.0)
                    u0 = work.tile([36, 512], F32, tag="u0")
                    nc.gpsimd.tensor_tensor(u0[32:36, :], sg0[32:36, :],
                                            h0n_prev[32:36, :], ALU.mult)
                    v0 = work.tile([36, 512], F32, tag="v0")
                    nc.vector.tensor_tensor(v0[32:36, :], n0[32:36, :], w0[32:36, :],
                                            ALU.mult)
                    h0n_new = hpool.tile([36, H], F32, tag="h0n")
                    nc.vector.tensor_tensor(h0n_new[32:36, :], v0[32:36, :],
                                            u0[32:36, :], ALU.add)

                    # === PE: h0T transposes ===
                    ps_h0T = ps_small[:, 32:32 + hkt * BL].rearrange(
                        "p (a b) -> p a b", a=hkt)
                    for hc in range(hkt):
                        nc.tensor.transpose(
                            ps_h0T[:, hc, :],
                            h0n_new[32:36, hc * 128:(hc + 1) * 128],
                            eyef[32:36, :], tile_position=(32, 0))
                    h0T_sb = hpool.tile([128, hkt, BL], BF16, tag="h0T")
                    nc.vector.tensor_copy(h0T_sb[:], ps_h0T)

                    # === PE: L1 i-part waves (from h0T) ===
                    for kt in range(hkt):
                        lhs = h0T_sb[:, kt, :]
                        nc.tensor.matmul(ps_g1[0:4, :], lhs, w1i_rz[:, kt, 0:512],
                                         start=False, stop=(kt == hkt - 1),
                                         tile_position=(0, 0))
                        nc.tensor.matmul(ps_g1[32:36, :], lhs,
                                         w1i_rz[:, kt, 512:1024],
                                         start=False, stop=(kt == hkt - 1),
                                         tile_position=(0, 32))
                        nc.tensor.matmul(ps_g1[96:100, :], lhs, w1i_n[:, kt, :],
                                         start=False, stop=(kt == hkt - 1),
                                         tile_position=(0, 96))

                    # === L1 elementwise ===
                    sg1 = work.tile([36, 512], BF16, tag="sg1")
                    nc.scalar.activation(sg1[:], ps_g1[0:36, :], AF.Sigmoid)
                    m1 = work.tile([36, 512], BF16, tag="m1")
                    nc.vector.tensor_tensor(m1[32:36, :], sg1[0:4, :],
                                            ps_g1[64:68, :], ALU.mult)
                    np1 = work.tile([36, 512], BF16, tag="np1")
                    nc.vector.tensor_tensor(np1[32:36, :], m1[32:36, :],
                                            ps_g1[96:100, :], ALU.add)
                    n1 = work.tile([36, 512], BF16, tag="n1")
                    nc.scalar.activation(n1[32:36, :], np1[32:36, :], AF.Tanh)
                    w1 = work.tile([36, 512], F32, tag="w1")
                    nc.scalar.activation(w1[32:36, :], sg1[32:36, :], AF.Identity,
                                         bias=1.0, scale=-1.0)
                    u1 = work.tile([36, 512], F32, tag="u1")
                    nc.gpsimd.tensor_tensor(u1[32:36, :], sg1[32:36, :],
                                            h1n_prev[32:36, :], ALU.mult)
                    v1 = work.tile([36, 512], F32, tag="v1")
                    nc.vector.tensor_tensor(v1[32:36, :], n1[32:36, :], w1[32:36, :],
                                            ALU.mult)
                    h1n_new = hpool.tile([36, H], F32, tag="h1n")
                    nc.vector.tensor_tensor(h1n_new[32:36, :], v1[32:36, :],
                                            u1[32:36, :], ALU.add)

                    # === PE: h1T transposes -> featsT h-part ===
                    ps_h1T = ps_small[:, 48:48 + hkt * BL].rearrange(
                        "p (a b) -> p a b", a=hkt)
                    for hc in range(hkt):
                        nc.tensor.transpose(
                            ps_h1T[:, hc, :],
                            h1n_new[32:36, hc * 128:(hc + 1) * 128],
                            eyef[32:36, :], tile_position=(32, 0))
                    nc.vector.tensor_copy(featsT[:, 0:hkt, tc0:tc1], ps_h1T)

                    h0T_prev = h0T_sb[:, :, :]
                    h0n_prev, h1n_prev = h0n_new, h1n_new

            # ---- phase B: projection + log-softmax ----
            with ExitStack() as bctx:
                bsing = bctx.enter_context(tc.tile_pool(name="bsing", bufs=1))
                wstr = bctx.enter_context(tc.tile_pool(name="wstr", bufs=3))
                escp = bctx.enter_context(tc.tile_pool(name="escp", bufs=2))
                otp = bctx.enter_context(tc.tile_pool(name="otp", bufs=6))
                stat = bctx.enter_context(tc.tile_pool(name="stat", bufs=1))
                ps_b = bctx.enter_context(tc.tile_pool(name="ps_b", bufs=4,
                                                       space="PSUM"))
                kt2 = 2 * hkt
                logits = bsing.tile([128, mch, nvt, VT], BF16)
                sums = bsing.tile([128, mch, nvt], F32)
                feats8 = bsing.tile([128, kt2, cfg.rows], FP8)
                nc.scalar.mul(feats8[:], featsT[:], SF)
                sinv = 1.0 / (SW * SF)

                for v in range(nvt):
                    wt = wstr.tile([128, kt2, 512], FP8, tag="wt")
                    nc.vector.memset(wt[:, :, VT:512], 0.0)
                    nc.sync.dma_start(wt[:, :, 0:VT],
                                      d_outWT[:, :, v * VT:(v + 1) * VT])
                    bt = wstr.tile([1, VT], BF16, tag="bt")
                    nc.sync.dma_start(bt[:], d_outb[:, v * VT:(v + 1) * VT])
                    for m in range(mch):
                        acc = ps_b.tile([128, 512], F32, tag="acc")
                        for kt in range(kt2 // 2):
                            nc.tensor.matmul(
                                acc[:],
                                feats8[:, 2 * kt:2 * kt + 2, m * 128:(m + 1) * 128],
                                wt[:, 2 * kt:2 * kt + 2, :],
                                start=(kt == 0), stop=False,
                                perf_mode=mybir.MatmulPerfMode.DoubleRow)
                        nc.tensor.matmul(acc[:, 0:VT], ones_1_128[:], bt[:],
                                         start=False, stop=True)
                        nc.vector.tensor_scalar_mul(logits[:, m, v, :],
                                                    acc[:, 0:VT], sinv)
                        esc = escp.tile([128, VT], BF16, tag="esc")
                        nc.scalar.activation(esc[:], acc[:, 0:VT], AF.Exp,
                                             scale=sinv,
                                             accum_out=sums[:, m, v:v + 1])

                nlz = stat.tile([128, mch], F32)
                for m in range(mch):
                    gsum = stat.tile([128, 1], F32, tag=f"gs{m}")
                    nc.vector.tensor_reduce(gsum[:], sums[:, m, :],
                                            mybir.AxisListType.X, ALU.add)
                    lnz = stat.tile([128, 1], F32, tag=f"ln{m}")
                    nc.scalar.activation(lnz[:], gsum[:], AF.Ln)
                    nc.vector.tensor_scalar_mul(nlz[:, m:m + 1], lnz[:], -1.0)

                for v in range(nvt):
                    for m in range(mch):
                        k = (v * mch + m) % 3
                        ot = otp.tile([128, VT], BF16, tag=f"ot{k}")
                        if k == 0:
                            nc.vector.tensor_scalar_add(ot[:], logits[:, m, v, :],
                                                        nlz[:, m:m + 1])
                        elif k == 1:
                            nc.scalar.activation(ot[:], logits[:, m, v, :],
                                                 AF.Identity, bias=nlz[:, m:m + 1])
                        else:
                            nc.gpsimd.tensor_scalar_add(ot[:], logits[:, m, v, :],
                                                        nlz[:, m:m + 1])
                        nc.sync.dma_start(
                            d_out[m * 128:(m + 1) * 128, v * VT:(v + 1) * VT],
                            ot[:])
        for _rep in range(reps):
            one_rep()
    return nc


# ----------------------------------------------------------------------------
# host-side prep
# ----------------------------------------------------------------------------

def _prep_core(cfg, inputs, lengths, final_hidden, hiddens, att_W, att_b,
               wdict, core):
    T, S, H, IN, V = cfg.T, cfg.S, cfg.H, cfg.IN, cfg.V
    st, hkt, xkt = cfg.st, cfg.hkt, cfg.xkt
    bs = slice(core * BL, (core + 1) * BL)
    bf = ml_dtypes.bfloat16

    hid_c = hiddens[bs]                                   # (BL, S, H)
    hid_l = np.ascontiguousarray(
        hid_c.reshape(BL, st, 128, H).transpose(2, 0, 1, 3)).astype(bf)
    x_c = inputs[bs]                                      # (BL, T, IN)
    xT = np.ascontiguousarray(
        x_c.transpose(2, 1, 0).reshape(xkt, 128, T * BL).transpose(1, 0, 2)
    ).astype(bf)
    mask = (np.arange(S)[None, :] < np.asarray(lengths)[bs, None]).astype(
        np.float32)
    aW_h = att_W[0, :H].astype(np.float32)
    aW_e = att_W[0, H:].astype(np.float32)
    encl = hid_c.astype(np.float32) @ aW_e + float(att_b[0])
    maskT = np.ascontiguousarray(
        mask.T.reshape(st, 128, BL).transpose(1, 0, 2)).astype(np.float32)
    enclT = np.ascontiguousarray(
        encl.T.reshape(st, 128, BL).transpose(1, 0, 2)).astype(np.float32)
    aWhT = aW_h.reshape(hkt, 128, 1).transpose(1, 0, 2).astype(bf)
    hn = final_hidden[:, bs, :].astype(np.float32)        # (2, BL, H)
    hT0 = np.zeros((128, 2 * hkt, BL), np.float32)
    for layer in range(2):
        hT0[:, layer * hkt:(layer + 1) * hkt, :] = (
            hn[layer].T.reshape(hkt, 128, BL).transpose(1, 0, 2))
    eyeb = np.zeros((128, BL), np.float32)
    for a in range(4):
        eyeb[32 * a:32 * a + BL, :] = np.eye(BL)
    in_map = dict(
        hid=np.ascontiguousarray(hid_l),
        xT=xT,
        maskT=maskT,
        enclT=enclT,
        aWhT=np.ascontiguousarray(aWhT),
        hT0=hT0.astype(bf),
        hnat0=hn,
        sel=np.kron(np.ones((st, 1), np.float32),
                    np.eye(BL, dtype=np.float32)).astype(bf),
        eyeb=eyeb.astype(bf),
        eyef=eyeb,
    )
    in_map.update(wdict)
    return in_map


def _prep_shared(cfg, att_W, out_W, out_b, W_ih0, W_hh0, b_ih0, b_hh0,
                 W_ih1, W_hh1, b_ih1, b_hh1):
    H, V = cfg.H, cfg.V
    hkt, xkt = cfg.hkt, cfg.xkt
    bf = ml_dtypes.bfloat16
    G2 = 2 * H

    def kt_layout(Wt, nkt):
        K, N = Wt.shape
        assert K == nkt * 128
        return np.ascontiguousarray(
            Wt.reshape(nkt, 128, N).transpose(1, 0, 2)).astype(bf)

    w = {}
    w["w0i_rz"] = kt_layout(W_ih0[:G2, :].T.astype(np.float32), xkt + hkt)
    w["w0i_n"] = kt_layout(W_ih0[G2:, :].T.astype(np.float32), xkt + hkt)
    w["w0h_rz"] = kt_layout(W_hh0[:G2, :].T.astype(np.float32), hkt)
    w["w0h_n"] = kt_layout(W_hh0[G2:, :].T.astype(np.float32), hkt)
    w["w1i_rz"] = kt_layout(W_ih1[:G2, :].T.astype(np.float32), hkt)
    w["w1i_n"] = kt_layout(W_ih1[G2:, :].T.astype(np.float32), hkt)
    w["w1h_rz"] = kt_layout(W_hh1[:G2, :].T.astype(np.float32), hkt)
    w["w1h_n"] = kt_layout(W_hh1[G2:, :].T.astype(np.float32), hkt)
    w["brz0"] = (b_ih0[:G2] + b_hh0[:G2]).reshape(1, G2).astype(bf)
    w["bin0"] = b_ih0[G2:].reshape(1, H).astype(bf)
    w["bhn0"] = b_hh0[G2:].reshape(1, H).astype(bf)
    w["brz1"] = (b_ih1[:G2] + b_hh1[:G2]).reshape(1, G2).astype(bf)
    w["bin1"] = b_ih1[G2:].reshape(1, H).astype(bf)
    w["bhn1"] = b_hh1[G2:].reshape(1, H).astype(bf)
    wf = out_W.T.astype(np.float32) * 256.0
    w["outWT"] = np.ascontiguousarray(
        wf.reshape(2 * hkt, 128, V).transpose(1, 0, 2)).astype(
        ml_dtypes.float8_e4m3fn)
    w["outb"] = (out_b * (256.0 * 16.0)).reshape(1, V).astype(bf)
    return w


_CACHED = {}


def _collect(inputs):
    g = lambda k: np.asarray(inputs[k], dtype=np.float32)
    return dict(
        inp=g("inputs"), lengths=np.asarray(inputs["lengths"]),
        final_hidden=g("final_hidden"), hiddens=g("hiddens"),
        att_W=g("att_W"), att_b=g("att_b"), out_W=g("out_W"), out_b=g("out_b"),
        W_ih0=g("W_ih0"), W_hh0=g("W_hh0"), b_ih0=g("b_ih0"), b_hh0=g("b_hh0"),
        W_ih1=g("W_ih1"), W_hh1=g("W_hh1"), b_ih1=g("b_ih1"), b_hh1=g("b_hh1"))


def _make_in_maps(cfg, a):
    wdict = _prep_shared(cfg, a["att_W"], a["out_W"], a["out_b"],
                         a["W_ih0"], a["W_hh0"], a["b_ih0"], a["b_hh0"],
                         a["W_ih1"], a["W_hh1"], a["b_ih1"], a["b_hh1"])
    return [
        _prep_core(cfg, a["inp"], a["lengths"], a["final_hidden"],
                   a["hiddens"], a["att_W"], a["att_b"], wdict, core)
        for core in range(NCORES)
    ]


def build_for_bench(**inputs):
    import os
    cfg = Cfg()
    a = _collect(inputs)
    nc = build(cfg, reps=int(os.environ.get("KREPS", "1")))
    if not nc.is_finalized():
        nc.finalize()
    return nc, _make_in_maps(cfg, a)


def assemble_output(results, inputs):
    cfg = Cfg()
    outs = []
    for c in range(NCORES):
        o = results[c]["out"].astype(np.float32).reshape(
            cfg.T, BL, cfg.V).transpose(1, 0, 2)
        outs.append(o)
    return np.concatenate(outs, axis=0)


def kernel(**inputs):
    cfg = Cfg()
    a = _collect({k: np.asarray(v) if not np.isscalar(v) else v
                  for k, v in inputs.items()})
    from concourse.bass_utils import run_bass_kernel_spmd
    if "nc" not in _CACHED:
        nc = build(cfg)
        if not nc.is_finalized():
            nc.finalize()
        _CACHED["nc"] = nc
    nc = _CACHED["nc"]
    in_maps = _make_in_maps(cfg, a)
    res = run_bass_kernel_spmd(nc, in_maps, list(range(NCORES)))
    return assemble_output(res.results, inputs)

